# revision 1
# baseline (speedup 1.0000x reference)
"""Canny edge detection (nn_Canny) on 8 Trainium2 NeuronCores.

Data-parallel: the batch dim (8) is sharded 1 image per core via pmap.
All stages are local stencils (<=5px halo), kernels are tiny replicated
constants, so no cross-device communication is needed.

This neuronx-cc build has a broken native-kernel registry, so any HLO
that pattern-matches into a "native kernel" (conv, arctan2, pooling)
aborts compilation.  Everything is therefore written as explicit
pad + shift + elementwise arithmetic:
  - the 3x3 convolutions are 9-term shifted weighted sums,
  - the angle buckets use ratio comparisons instead of arctan2/mod,
  - the 5x5 hysteresis max-pool is a separable shift-max chain.
auto-cast is disabled so everything stays fp32 (bf16 would flip the
50/80 threshold comparisons and the NMS float-equality test).
"""

import os

_flags = os.environ.get("NEURON_CC_FLAGS", "")
if "--auto-cast" not in _flags:
    os.environ["NEURON_CC_FLAGS"] = (_flags + " --auto-cast none").strip()

import jax
import jax.numpy as jnp
import numpy as np

THRESH_MIN = 50.0
THRESH_MAX = 80.0
TRACK_ITERS = 3

N_CORES = 8
NEG_INF = np.float32(-np.inf)

_ANGLE_OFFSETS = [
    [(0, -1), (0, 1)],    # 0 deg   (center handled implicitly)
    [(-1, 1), (1, -1)],   # 45 deg
    [(-1, 0), (1, 0)],    # 90 deg
    [(-1, -1), (1, 1)],   # 135 deg
]


def _shift2d(img, dy, dx, fill):
    """img shifted so out[y, x] = img[y + dy, x + dx], out-of-range = fill."""
    H, W = img.shape
    r = max(abs(dy), abs(dx), 1)
    pad = jnp.pad(img, ((r, r), (r, r)), constant_values=fill)
    return pad[r + dy : r + dy + H, r + dx : r + dx + W]


def _conv3x3(img, k):
    """'SAME' zero-padded 3x3 convolution of a 2D image.

    Matches lax.conv semantics: out[y,x] = sum_{i,j} k[i,j]*img[y+i-1, x+j-1].
    """
    H, W = img.shape
    pad = jnp.pad(img, ((1, 1), (1, 1)))
    out = None
    for i in range(3):
        for j in range(3):
            term = k[i, j] * pad[i : i + H, j : j + W]
            out = term if out is None else out + term
    return out


def _canny_2d(img, gk, skx, sky):
    """img: (H, W) fp32. gk: (3,3) gaussian. skx/sky: (3,3) sobel kernels."""
    dt = img.dtype
    sm = _conv3x3(img, gk)
    gx = _conv3x3(sm, skx)
    gy = _conv3x3(sm, sky)
    grad_mag = jnp.clip(jnp.sqrt(gx * gx + gy * gy), 0.0, 255.0)

    # Angle buckets without arctan2/mod.  With
    # theta = (atan2(gy,gx)*180/pi + 90) mod 180, tan(theta) = -gx/gy, so
    #   m0: theta in [157.5,180)u[0,22.5]  <=>  |gx| <= tan(22.5)*|gy|
    #   m2: theta in [67.5,112.5)          <=>  |gx| >= tan(67.5)*|gy|
    #   m1: else with tan(theta) > 0       <=>  gx*gy < 0;  m3: remaining.
    # (At gx=gy=0, m0 and m2 both fire vs. reference's m2-only; harmless
    # since grad_mag=0 makes those responses identical anyway.)
    t1 = np.float32(np.tan(np.deg2rad(22.5)))
    t2 = np.float32(np.tan(np.deg2rad(67.5)))
    ax, ay = jnp.abs(gx), jnp.abs(gy)
    m0 = ax <= t1 * ay
    m2 = ax >= t2 * ay
    mid = (~m0) & (~m2)
    neg = (gx * gy) < 0.0
    m1 = mid & neg
    m3 = mid & (~neg)

    masks = [m.astype(dt) for m in (m0, m1, m2, m3)]
    responses = [m * grad_mag for m in masks]

    # Directional dilation (reference pads with -inf; since the window
    # always contains the center and responses >= 0, the dilated value
    # equals the reference's) + float-equality NMS.
    any_eq = None
    for resp, offs in zip(responses, _ANGLE_OFFSETS):
        m = resp
        for dy, dx in offs:
            m = jnp.maximum(m, _shift2d(resp, dy, dx, NEG_INF))
        eq = (m == resp).astype(dt)
        any_eq = eq if any_eq is None else jnp.maximum(any_eq, eq)

    edge_candidates = grad_mag * any_eq
    strong = (edge_candidates >= THRESH_MAX).astype(dt)
    weak = (
        (edge_candidates >= THRESH_MIN) & (edge_candidates < THRESH_MAX)
    ).astype(dt)

    # 3 iterations of: strong |= weak & (5x5 maxpool of strong), as a
    # separable shift-max chain (reduce_window also trips the broken
    # native-kernel path).  -inf pad matches reference; the center tap
    # makes the pooled value >= 0 everywhere regardless.
    for _ in range(TRACK_ITERS):
        v = strong
        for d in (-2, -1, 1, 2):
            v = jnp.maximum(v, _shift2d(strong, d, 0, NEG_INF))
        p = v
        for d in (-2, -1, 1, 2):
            p = jnp.maximum(p, _shift2d(v, 0, d, NEG_INF))
        strong = jnp.clip(strong + weak * p, 0.0, 1.0)
    return strong.astype(dt)


def _per_core(x_img, gaussian_kernel, sobel_kernel):
    # x_img: (H, W, 1) — one image on this core.
    gk = gaussian_kernel[:, :, 0, 0]
    skx = sobel_kernel[:, :, 0, 0]
    sky = sobel_kernel[:, :, 0, 1]
    return _canny_2d(x_img[:, :, 0], gk, skx, sky)[:, :, None]


_pmapped = None


def _get_pmapped():
    global _pmapped
    if _pmapped is None:
        _pmapped = jax.pmap(
            _per_core,
            in_axes=(0, None, None),
            devices=jax.devices()[:N_CORES],
        )
    return _pmapped


def kernel(x, gaussian_kernel, sobel_kernel):
    x = np.asarray(x, dtype=np.float32)
    gk = np.asarray(gaussian_kernel, dtype=np.float32)
    sk = np.asarray(sobel_kernel, dtype=np.float32)
    out = _get_pmapped()(x, gk, sk)
    return np.asarray(out).astype(np.float32)

